# revision 28
# baseline (speedup 1.0000x reference)
"""Distributed GPT-2 attention block for one TRN2 chip (8 NeuronCores).

Sharding: core c -> (batch b = c//2, head-group g = c%2).  Each core computes
the QKV projection for its 8 heads (Megatron column split), full causal
attention for those heads, and a partial output projection (row split of
c_proj_w).  The host sums the two partials per batch and adds c_proj_b.
No cross-core collectives are needed.

Device kernel highlights:
  - host passes x pre-transposed (xT), so Q^T, K^T (head-dim-major) and V
    (seq-major) all come straight out of matmuls with zero on-chip
    transposes;
  - scores are computed transposed (s^T[k,q] = K Q^T), diagonal-crossing
    tiles masked additively in PSUM (-3e4 underflows to exactly 0 after
    exp), exp'd on ScalarE;
  - fully-masked 128x512 score tiles are skipped, and diagonal-crossing
    tiles are computed/exp'd/consumed only on their live column range, so
    masked columns cost no PE, ACT or DVE work at all;
  - attn^T = V_aug^T exp^T directly yields the projection-ready layout;
    V_aug columns 64:128 are all-ones, so the same matmul lands the softmax
    denominator pre-broadcast across PSUM partitions 64:128 (matmul time
    scales with streamed rows, not output partitions) -- normalization is
    then just a DVE reciprocal + multiply, no PE in the chain;
  - emission is software-pipelined: PE executes in program order, so AV
    matmuls trail their Exp by two score-steps, and QKV-projection /
    out-projection matmul units (own PSUM ring) are interleaved into the
    attention stream to keep PE fed while ScalarE works through the Exps
    (attention alone is ACT-bound; the whole kernel is PE-bound);
  - weights and the V_aug ones columns are loaded once outside the `reps`
    loop, and K^T/V_aug are double-buffered across reps, so repeat-unrolled
    NEFFs pipeline cleanly and measure the true marginal per-iteration
    cost; a `repout` witness output counts executed reps;
  - matmul inputs are bf16 (f32 PSUM accumulation), cast on host.  A
    float32r (tf32-like) variant exists behind dtype="f32r" -- it passes the
    BIR verifier and CoreSim but reliably wedges this fleet's PE
    (NRT_EXEC_UNIT_UNRECOVERABLE), so bf16 is the shipped default.

Measured (TimelineSim cost model): 307 us single-shot, 259 us/rep marginal
at reps>=4.  HW (reps-delta, axon): ~245 us/rep vs ~386 us for the previous
sequential-phase kernel.
"""
import sys

if "/opt/trn_rl_repo" not in sys.path:
    sys.path.insert(0, "/opt/trn_rl_repo")

import numpy as np

_B, _S, _D = 4, 2048, 1024
_H, _HL, _DH, _DL = 16, 8, 64, 512
_P = 128
_NCORES = 8
_NQC = _S // 512      # 4  q-chunks of 512
_NKT = _S // _P       # 16 k-tiles of 128
_NDC = _D // _P       # 8  contraction chunks for QKV
_NLC = _DL // _P      # 4  contraction chunks for out-proj
_MASK_ADD = -30000.0  # exp(score + _MASK_ADD) == 0.0 exactly in fp32


def _round_f32r(x: np.ndarray) -> np.ndarray:
    """Round fp32 to fp32r (11-bit mantissa, low 12 bits zero), RTN-even."""
    u = np.ascontiguousarray(x, dtype=np.float32).view(np.uint32)
    low = u & np.uint32(0xFFF)
    u = u & np.uint32(0xFFFFF000)
    round_up = (low > 0x800) | ((low == 0x800) & (((u >> 12) & 1) == 1))
    return (u + (round_up.astype(np.uint32) << 12)).view(np.float32)


def build_graph(phases=(1, 2, 3), mm_bufs=2, expt_bufs=3, at_bufs=2, dtype="bf16", p1w=512, xt_bufs=3, inline_proj=True, reps=1, out_bf16=True, exp_pair=True, exp_width=None, ctl=False, p1_bufs=2, qk_dve=False):
    import concourse.bass as bass
    import concourse.tile as tile
    import concourse.mybir as mybir
    from concourse import bacc
    from concourse.bass import ts

    F32R = mybir.dt.float32r if dtype == "f32r" else mybir.dt.bfloat16
    F32 = mybir.dt.float32
    AF = mybir.ActivationFunctionType
    ALU = mybir.AluOpType
    P = _P

    nc = bacc.Bacc("TRN2", target_bir_lowering=False, debug=False,
                   enable_asserts=True, num_devices=_NCORES)
    xt = nc.dram_tensor("xt", [_D, _S], F32R, kind="ExternalInput")
    wq = nc.dram_tensor("wq", [_D, _DL], F32R, kind="ExternalInput")
    wk = nc.dram_tensor("wk", [_D, _DL], F32R, kind="ExternalInput")
    wv = nc.dram_tensor("wv", [_D, _DL], F32R, kind="ExternalInput")
    wp = nc.dram_tensor("wp", [_DL, _D], F32R, kind="ExternalInput")
    bq = nc.dram_tensor("bq", [_DL], F32, kind="ExternalInput")
    bk = nc.dram_tensor("bk", [_DL], F32, kind="ExternalInput")
    bv = nc.dram_tensor("bv", [_DL], F32R, kind="ExternalInput")
    trineg = nc.dram_tensor("trineg", [P, P], F32, kind="ExternalInput")
    onesr = nc.dram_tensor("onesr", [1, P], F32R, kind="ExternalInput")
    OUT_DT = mybir.dt.bfloat16 if out_bf16 else F32
    out = nc.dram_tensor("out", [_S, _D], OUT_DT, kind="ExternalOutput")
    # tiny witness output: counts executed reps so repeat-unrolled timing
    # NEFFs can prove they really ran R iterations
    repout = nc.dram_tensor("repout", [1], F32, kind="ExternalOutput")

    with tile.TileContext(nc) as tc:
        with tc.tile_pool(name="const", bufs=1) as constp, \
             tc.tile_pool(name="persist", bufs=1) as persist, \
             tc.tile_pool(name="work", bufs=1) as work, \
             tc.tile_pool(name="psS", bufs=mm_bufs, space="PSUM") as psS, \
             tc.tile_pool(name="psA", bufs=at_bufs, space="PSUM") as psA:

            if ctl:  # control variant: identical graph, swapped const order
                ones_sb = constp.tile([1, P], F32R)
                nc.sync.dma_start(ones_sb[:], onesr.ap())
                trineg_sb = constp.tile([P, P], F32)
                nc.sync.dma_start(trineg_sb[:], trineg.ap())
            else:
                trineg_sb = constp.tile([P, P], F32)
                nc.sync.dma_start(trineg_sb[:], trineg.ap())
                ones_sb = constp.tile([1, P], F32R)
                nc.sync.dma_start(ones_sb[:], onesr.ap())
            bq_sb = constp.tile([P, _NLC], F32)
            nc.sync.dma_start(bq_sb[:], bq.ap().rearrange("(o p) -> p o", p=P))
            bk_sb = constp.tile([P, _NLC], F32)
            nc.sync.dma_start(bk_sb[:], bk.ap().rearrange("(o p) -> p o", p=P))
            bvr_sb = constp.tile([1, _DL], F32R)
            nc.sync.dma_start(bvr_sb[:], bv.ap()[None, :])


            # Double-buffer the per-iteration activations across reps so a
            # repeat-unrolled NEFF pipelines cleanly (next rep's QKV writes
            # never WAR-serialize against this rep's attention reads).
            nbuf = 2 if reps > 1 else 1
            # QT is only read by the q-chunk that owns it, so the next rep's
            # rewrite of chunk 0 never races this rep's tail; KT/Vaug are
            # read in full by every q-chunk and need real double-buffering.
            QT_sb = [persist.tile([P, _NLC, _S], F32R, name="QT0")] * nbuf
            KT_sb = [persist.tile([P, _NLC, _S], F32R, name=f"KT{i}")
                     for i in range(nbuf)]
            # Vaug cols 64:128 are all-ones: the AV matmul then lands the
            # softmax denominator already broadcast across partitions 64:128
            # of at_ps, for free (matmul cost scales with streamed rows, not
            # output partitions).
            Vaug_sb = [persist.tile([P, _NKT, _HL, P], F32R,
                                    name=f"Vaug{i}") for i in range(nbuf)]

            xt_r = xt.ap().rearrange("(po p) s -> p po s", p=P)
            wq_r = wq.ap().rearrange("(po p) f -> p po f", p=P)
            wk_r = wk.ap().rearrange("(po p) f -> p po f", p=P)
            wv_r = wv.ap().rearrange("(po p) f -> p po f", p=P)

            # weights + Vaug ones column are invariant across reps
            with tc.tile_pool(name="wqkv", bufs=1) as wqkvp, \
                 tc.tile_pool(name="xtp", bufs=2) as xtp, \
                 tc.tile_pool(name="psP", bufs=p1_bufs, space="PSUM") as psP:
                wq_sb = wqkvp.tile([P, _NDC, _DL], F32R)
                nc.sync.dma_start(wq_sb[:], wq_r)
                wk_sb = wqkvp.tile([P, _NDC, _DL], F32R)
                nc.sync.dma_start(wk_sb[:], wk_r)
                wv_sb = wqkvp.tile([P, _NDC, _DL], F32R)
                nc.sync.dma_start(wv_sb[:], wv_r)
                wp_sb = wqkvp.tile([P, _NLC, _D], F32R)
                nc.sync.dma_start(wp_sb[:],
                                  wp.ap().rearrange("(po p) f -> p po f", p=P))
                aT_sb = wqkvp.tile([P, _NLC, _S], F32R)
                cnt_sb = wqkvp.tile([1, 1], F32)
                nc.vector.memset(cnt_sb[:], 0.0)
                for b in range(nbuf):
                    nc.gpsimd.memset(Vaug_sb[b][:, :, :, _DH:P], 1.0)

                # ---- emission machinery ----------------------------------
                # PE executes in program order, so consumers of cross-engine
                # results (AV matmuls behind Exp, the rank-1 denominator
                # broadcast behind the DVE reciprocal) are emitted a few
                # score-steps late, and independent QKV-projection /
                # out-projection matmul units (own PSUM ring) are interleaved
                # into the attention stream to keep PE fed while ScalarE
                # works through the Exps.
                pending = []  # (due_step, seq, fn)
                seqno = [0]
                step = [0]

                def emit_at(due, fn):
                    pending.append((due, seqno[0], fn))
                    seqno[0] += 1

                def flush(now):
                    pending.sort(key=lambda x: (x[0], x[1]))
                    while pending and pending[0][0] <= now:
                        pending.pop(0)[2]()

                def p1_units(j, b):
                    """Phase-1 PE units for the 512-wide chunk j (q-chunk and
                    k-tile range alike), writing buffer set b."""
                    jsl = slice(512 * j, 512 * (j + 1))
                    xt_c = xtp.tile([P, _NDC, 512], F32R, name=f"xtc{j}",
                                    tag="xtc", bufs=xt_bufs)
                    units = []

                    def dma(xt_c=xt_c, jsl=jsl):
                        nc.sync.dma_start(xt_c[:], xt_r[:, :, jsl])
                    units.append(dma)

                    for w_sb, dstT, b_sb in ((wq_sb, QT_sb[b], bq_sb),
                                             (wk_sb, KT_sb[b], bk_sb)):
                        for mi in range(_NLC):
                            def qk(w_sb=w_sb, dstT=dstT, b_sb=b_sb, mi=mi,
                                   xt_c=xt_c, jsl=jsl):
                                ps = psP.tile([P, 512], F32, name="qkps",
                                              tag="p1")
                                for kt in range(_NDC):
                                    nc.tensor.matmul(ps[:],
                                                     w_sb[:, kt, ts(mi, P)],
                                                     xt_c[:, kt, :],
                                                     start=(kt == 0),
                                                     stop=(kt == _NDC - 1))
                                if qk_dve:
                                    nc.vector.tensor_scalar_add(
                                        dstT[:, mi, jsl], ps[:],
                                        b_sb[:, mi:mi + 1])
                                else:
                                    nc.scalar.activation(dstT[:, mi, jsl],
                                                         ps[:], AF.Identity,
                                                         bias=b_sb[:, mi:mi + 1])
                            units.append(qk)
                    for si in range(4):
                        t = 4 * j + si
                        def vv(t=t, si=si, xt_c=xt_c, b=b):
                            ps = psP.tile([P, 512], F32, name="vps", tag="p1")
                            for kt in range(_NDC):
                                nc.tensor.matmul(ps[:], xt_c[:, kt, ts(si, P)],
                                                 wv_sb[:, kt, :],
                                                 start=(kt == 0), stop=False)
                            # + bias via rank-1 ones^T @ bv
                            nc.tensor.matmul(ps[:], ones_sb[:, :], bvr_sb[:],
                                             start=False, stop=True)
                            nc.vector.tensor_copy(
                                Vaug_sb[b][:, t, :, 0:_DH],
                                ps.rearrange("p (h d) -> p h d", h=_HL))
                        units.append(vv)
                    return units

                def proj_units(qj):
                    """Out-projection PE units for aT q-columns of chunk qj."""
                    units = []
                    for mi in range(4 * qj, 4 * qj + 4):
                        for nj in range(2):
                            def pj(mi=mi, nj=nj):
                                nsl = slice(512 * nj, 512 * (nj + 1))
                                ps = psP.tile([P, 512], F32, name="ops",
                                              tag="p1")
                                for kt in range(_NLC):
                                    nc.tensor.matmul(ps[:],
                                                     aT_sb[:, kt, ts(mi, P)],
                                                     wp_sb[:, kt, nsl],
                                                     start=(kt == 0),
                                                     stop=(kt == _NLC - 1))
                                o_sb = work.tile([P, 512], OUT_DT, name="osb",
                                                 tag="osb", bufs=3)
                                nc.vector.tensor_copy(o_sb[:], ps[:])
                                nc.sync.dma_start(out.ap()[ts(mi, P), nsl],
                                                  o_sb[:])
                            units.append(pj)
                    return units

                def attention(qj, b, fills):
                    """Attention for q-chunk qj reading buffer set b, with
                    independent PE units from `fills` interleaved."""
                    qsl = slice(512 * qj, 512 * (qj + 1))
                    nkc = 4 * qj + 4
                    nsteps = _HL * (nkc // 2)
                    stride = max(1, nsteps // max(1, len(fills)))
                    s_idx = 0
                    for h in range(_HL):
                        rows = slice(64 * (h % 2), 64 * (h % 2) + 64)
                        po = h // 2
                        at_ps = psA.tile([P, 512], F32, name=f"at{h}_{qj}",
                                         tag="at", bufs=at_bufs)
                        for kp in range(nkc // 2):
                            # two score tiles share one 2-bank PSUM tile;
                            # diagonal-crossing tiles live only on cols >= off
                            s_ps = psS.tile([P, 1024], F32, name="sps",
                                            tag="mm", bufs=mm_bufs)
                            offs = []
                            for half in range(2):
                                ki = 2 * kp + half
                                d = ki - 4 * qj
                                off = P * max(d, 0)
                                offs.append(off)
                                o = 512 * half
                                nc.tensor.matmul(
                                    s_ps[:, o + off:o + 512],
                                    KT_sb[b][rows, po, ts(ki, P)],
                                    QT_sb[b][rows, po,
                                             512 * qj + off:512 * (qj + 1)],
                                    start=True, stop=True)
                                if d >= 0:  # diagonal-crossing tile
                                    nc.vector.tensor_tensor(
                                        s_ps[:, o + off:o + off + P],
                                        s_ps[:, o + off:o + off + P],
                                        trineg_sb[:], ALU.add)
                            expT = work.tile([P, 1024], F32R, name="expT",
                                             tag="expT", bufs=expt_bufs)
                            if offs[0] == 0 and 2 * kp + 1 < 4 * qj:
                                # both tiles fully live: one wide Exp
                                nc.scalar.activation(expT[:], s_ps[:], AF.Exp)
                            else:
                                for half in range(2):
                                    o = 512 * half
                                    sl = slice(o + offs[half], o + 512)
                                    nc.scalar.activation(expT[:, sl],
                                                         s_ps[:, sl], AF.Exp)

                            def av(at_ps=at_ps, expT=expT, kp=kp, offs=offs,
                                   h=h, nkc=nkc, b=b):
                                for half in range(2):
                                    ki = 2 * kp + half
                                    off = offs[half]
                                    o = 512 * half
                                    nc.tensor.matmul(
                                        at_ps[:, off:512],
                                        Vaug_sb[b][:, ki, h, :],
                                        expT[:, o + off:o + 512],
                                        start=(ki == 0),
                                        stop=(ki == nkc - 1))
                            emit_at(step[0] + 2, av)
                            step[0] += 1
                            flush(step[0])
                            if fills and s_idx % stride == stride - 1:
                                fills.pop(0)()
                            s_idx += 1

                        def norm(at_ps=at_ps, rows=rows, po=po, qsl=qsl):
                            # denominator sits pre-broadcast in partitions
                            # 64:128; reciprocal + multiply, no PE involved
                            rec_sb = work.tile([_DH, 512], F32, name="recsb",
                                               tag="rec", bufs=2)
                            nc.vector.reciprocal(rec_sb[:], at_ps[_DH:P, :])
                            nc.vector.tensor_tensor(aT_sb[rows, po, qsl],
                                                    at_ps[0:_DH, :],
                                                    rec_sb[:], ALU.mult)
                        emit_at(step[0] + 2, norm)
                    # anything this block didn't absorb runs now
                    while fills:
                        fills.pop(0)()

                # ---- main schedule ---------------------------------------
                for _rep in range(reps):
                    b = _rep % nbuf
                    if 1 in phases and 2 not in phases:
                        for j in range(_NQC):
                            for u in p1_units(j, b):
                                u()
                    elif 1 in phases and _rep == 0:
                        for u in p1_units(0, b):
                            u()
                    if 2 in phases:
                        for qj in range(_NQC):
                            fills = []
                            if 1 in phases:
                                if qj < _NQC - 1:
                                    fills += p1_units(qj + 1, b)
                                elif _rep + 1 < reps:
                                    fills += p1_units(0, (_rep + 1) % nbuf)
                            if 3 in phases:
                                if qj > 0:
                                    fills += proj_units(qj - 1)
                                elif _rep > 0:
                                    fills += proj_units(_NQC - 1)
                            attention(qj, b, fills)
                    nc.vector.tensor_scalar_add(cnt_sb[:], cnt_sb[:], 1.0)
                flush(10 ** 9)
                if 3 in phases:
                    for u in proj_units(_NQC - 1):
                        u()
                nc.sync.dma_start(repout.ap()[None, :], cnt_sb[:])

    nc.compile()
    return nc


def make_in_maps(x, c_attn_w, c_attn_b, c_proj_w, dtype="f32r"):
    if dtype == "f32r":
        cvt = _round_f32r
    else:
        import ml_dtypes
        cvt = lambda a: np.ascontiguousarray(a, dtype=np.float32).astype(ml_dtypes.bfloat16)
    x = np.asarray(x, dtype=np.float32)
    c_attn_w = np.asarray(c_attn_w, dtype=np.float32)
    c_attn_b = np.asarray(c_attn_b, dtype=np.float32)
    c_proj_w = np.asarray(c_proj_w, dtype=np.float32)

    k = np.arange(_P)[:, None]
    q = np.arange(_P)[None, :]
    trineg = np.where(k <= q, 0.0, _MASK_ADD).astype(np.float32)
    ones = np.ones((1, _P), dtype=np.float32)

    xts = [cvt(np.ascontiguousarray(x[b].T)) for b in range(_B)]
    in_maps = []
    for c in range(_NCORES):
        b, g = divmod(c, 2)
        lo = g * _DL
        in_maps.append({
            "xt": xts[b],
            "wq": cvt(c_attn_w[:, lo:lo + _DL] * 0.125),
            "wk": cvt(c_attn_w[:, _D + lo:_D + lo + _DL]),
            "wv": cvt(c_attn_w[:, 2 * _D + lo:2 * _D + lo + _DL]),
            "wp": cvt(c_proj_w[lo:lo + _DL, :]),
            "bq": np.ascontiguousarray(c_attn_b[lo:lo + _DL] * 0.125),
            "bk": np.ascontiguousarray(c_attn_b[_D + lo:_D + lo + _DL]),
            "bv": cvt(c_attn_b[2 * _D + lo:2 * _D + lo + _DL]),
            "trineg": trineg,
            "onesr": cvt(ones),

        })
    return in_maps


_RUNNER = None


def _make_runner(dtype="f32r", **build_kw):
    """Build graph once and return (nc, run_fn) with a persistent jitted
    executor (mirrors bass2jax.run_bass_via_pjrt's multi-core path, but
    hoists the jit so repeated calls don't recompile)."""
    import jax
    import concourse.mybir as mybir
    from concourse import bass2jax
    from jax.experimental.shard_map import shard_map
    from jax.sharding import Mesh, PartitionSpec

    nc = build_graph(dtype=dtype, **build_kw)
    bass2jax.install_neuronx_cc_hook()

    partition_name = (nc.partition_id_tensor.name
                      if nc.partition_id_tensor else None)
    in_names, out_names, out_avals, zero_outs = [], [], [], []
    for alloc in nc.m.functions[0].allocations:
        if not isinstance(alloc, mybir.MemoryLocationSet):
            continue
        name = alloc.memorylocations[0].name
        if alloc.kind == "ExternalInput":
            if name != partition_name:
                in_names.append(name)
        elif alloc.kind == "ExternalOutput":
            out_names.append(name)
            shape = tuple(alloc.tensor_shape)
            dtype = mybir.dt.np(alloc.dtype)
            out_avals.append(jax.core.ShapedArray(shape, dtype))
            zero_outs.append(np.zeros(shape, dtype))
    n_params = len(in_names)
    n_outs = len(out_avals)
    all_in_names = list(in_names) + list(out_names)
    if partition_name is not None:
        all_in_names.append(partition_name)
    donate = tuple(range(n_params, n_params + n_outs))

    def _body(*args):
        operands = list(args)
        if partition_name is not None:
            operands.append(bass2jax.partition_id_tensor())
        outs = bass2jax._bass_exec_p.bind(
            *operands,
            out_avals=tuple(out_avals),
            in_names=tuple(all_in_names),
            out_names=tuple(out_names),
            lowering_input_output_aliases=(),
            sim_require_finite=True,
            sim_require_nnan=True,
            nc=nc,
        )
        return tuple(outs)

    devices = jax.devices()[:_NCORES]
    mesh = Mesh(np.asarray(devices), ("core",))
    sharded = jax.jit(
        shard_map(_body, mesh=mesh,
                  in_specs=(PartitionSpec("core"),) * (n_params + n_outs),
                  out_specs=(PartitionSpec("core"),) * n_outs,
                  check_rep=False),
        donate_argnums=donate, keep_unused=True)

    def run(in_maps, time_iters=0):
        per_core = [[np.asarray(m[name]) for name in in_names] for m in in_maps]
        concat_in = [np.concatenate([per_core[c][i] for c in range(_NCORES)],
                                    axis=0) for i in range(n_params)]
        concat_zeros = [np.zeros((_NCORES * z.shape[0], *z.shape[1:]), z.dtype)
                        for z in zero_outs]
        out_arrs = sharded(*concat_in, *concat_zeros)
        jax.block_until_ready(out_arrs)
        walls = None
        if time_iters:
            call = make_timer(in_maps)
            walls = [call() for _ in range(time_iters)]
        outs = [{name: np.asarray(out_arrs[i]).reshape(_NCORES,
                                                       *out_avals[i].shape)[c]
                 for i, name in enumerate(out_names)}
                for c in range(_NCORES)]
        return outs, walls

    def make_timer(in_maps):
        """Return a closure performing one blocking timed execution with
        device-resident inputs and pre-built donated zero outputs."""
        import time
        from jax.sharding import NamedSharding
        shard = NamedSharding(mesh, PartitionSpec("core"))
        per_core = [[np.asarray(m[name]) for name in in_names] for m in in_maps]
        concat_in = [np.concatenate([per_core[c][i] for c in range(_NCORES)],
                                    axis=0) for i in range(n_params)]
        dev_in = [jax.device_put(a, shard) for a in concat_in]
        zshapes = [((_NCORES * z.shape[0], *z.shape[1:]), z.dtype)
                   for z in zero_outs]
        mk_zeros = jax.jit(
            lambda: tuple(jax.numpy.zeros(s, d) for s, d in zshapes),
            out_shardings=tuple(shard for _ in zshapes))
        jax.block_until_ready(dev_in)

        def call():
            cz = mk_zeros()
            jax.block_until_ready(cz)
            t0 = time.perf_counter()
            res = sharded(*dev_in, *cz)
            jax.block_until_ready(res)
            return time.perf_counter() - t0

        return call

    run.make_timer = make_timer
    return nc, run


_DTYPE = "bf16"


def get_runner():
    global _RUNNER
    if _RUNNER is None:
        _RUNNER = _make_runner(dtype=_DTYPE)
    return _RUNNER


def kernel(x, c_attn_w, c_attn_b, c_proj_w, c_proj_b):
    _, run = get_runner()
    in_maps = make_in_maps(x, c_attn_w, c_attn_b, c_proj_w, dtype=_DTYPE)
    outs, _ = run(in_maps)
    c_proj_b = np.asarray(c_proj_b, dtype=np.float32)
    res = np.empty((_B, _S, _D), dtype=np.float32)
    for b in range(_B):
        res[b] = (np.asarray(outs[2 * b]["out"], dtype=np.float32)
                  + np.asarray(outs[2 * b + 1]["out"], dtype=np.float32)
                  + c_proj_b[None, :])
    return res

